# revision 4
# baseline (speedup 1.0000x reference)
"""Triangular pairwise channel product on 8 Trainium2 NeuronCores.

out[b,h,w,k] = x[b,h,w,i_k] * x[b,h,w,j_k]  for the C*(C-1)/2 pairs
(i<j) in row-major (np.triu_indices) order.

Sharding: pure data parallel over batch — core c takes x[2c:2c+2].
Per core the 2*64*64 = 8192 spatial positions map to 128 SBUF
partitions (b_loc*64+h) x 64 groups (w).  For each group-block of G=8
positions, block i of the output (pairs (i, i+1..63)) is one fp32
tensor_tensor multiply whose first operand is x[:, :, i] broadcast via
a step-0 access pattern — 63 DVE ops cover all 2016 output channels
with per-partition-contiguous stores back to HBM.
"""

import numpy as np

import concourse.bacc as bacc
import concourse.bass as bass
import concourse.mybir as mybir
import concourse.tile as tile
from concourse.bass_utils import run_bass_kernel_spmd

B, H, W, C = 16, 64, 64, 64
K = C * (C - 1) // 2  # 2016
N_CORES = 8
BP = B // N_CORES  # batch rows per core
P = BP * H         # 128 SBUF partitions
G_TOTAL = W        # position groups per partition
G = 8              # groups processed per iteration
FP = mybir.dt.float32

_nc_cache = None


def build_bass() -> bass.Bass:
    # Bacc (not plain Bass): its compile() pipeline runs
    # generate_event_semaphores, which splits multi-wait instructions to
    # satisfy the TRN2 1-wait-per-instruction codegen limit.
    nc = bacc.Bacc(
        "TRN2",
        target_bir_lowering=False,
        debug=False,
        num_devices=N_CORES,
    )
    x = nc.dram_tensor("x", [P, G_TOTAL, C], FP, kind="ExternalInput")
    y = nc.dram_tensor("y", [P, G_TOTAL, K], FP, kind="ExternalOutput")

    with tile.TileContext(nc) as tc:
        with (
            tc.tile_pool(name="xin", bufs=2) as xpool,
            tc.tile_pool(name="out", bufs=2) as opool,
        ):
            for it in range(G_TOTAL // G):
                xt = xpool.tile([P, G, C], FP)
                nc.sync.dma_start(out=xt[:], in_=x[:, it * G : (it + 1) * G, :])

                ot = opool.tile([P, G, K], FP)
                ro = 0
                for i in range(C - 1):
                    w = C - 1 - i
                    a = xt[:, :, i : i + 1].broadcast_to([P, G, w])
                    b = xt[:, :, i + 1 : C]
                    nc.vector.tensor_mul(ot[:, :, ro : ro + w], a, b)
                    ro += w

                nc.sync.dma_start(out=y[:, it * G : (it + 1) * G, :], in_=ot[:])

    nc.finalize()
    return nc


def make_in_maps(x: np.ndarray) -> list[dict[str, np.ndarray]]:
    x = np.ascontiguousarray(x, dtype=np.float32)
    return [
        {"x": x[c * BP : (c + 1) * BP].reshape(P, G_TOTAL, C)} for c in range(N_CORES)
    ]


def kernel(**inputs: np.ndarray) -> np.ndarray:
    global _nc_cache
    if _nc_cache is None:
        _nc_cache = build_bass()
    res = run_bass_kernel_spmd(
        _nc_cache, make_in_maps(inputs["inputs"]), list(range(N_CORES))
    ).results
    return np.concatenate(
        [res[c]["y"].reshape(BP, H, W, K) for c in range(N_CORES)], axis=0
    )


# revision 5
# speedup vs baseline: 1.3109x; 1.3109x over previous
"""Triangular pairwise channel product on 8 Trainium2 NeuronCores.

out[b,h,w,k] = x[b,h,w,i_k] * x[b,h,w,j_k]  for the C*(C-1)/2 pairs
(i<j) in row-major (np.triu_indices) order.

Sharding: pure data parallel over batch — core c takes x[2c:2c+2].
Per core the 2*64*64 = 8192 spatial positions map to 128 SBUF
partitions (b_loc*64+h) x 64 groups (w).  For each group-block of G=8
positions, block i of the output (pairs (i, i+1..63)) is one fp32
tensor_tensor multiply whose first operand is x[:, :, i] broadcast via
a step-0 access pattern — 63 DVE ops cover all 2016 output channels
with per-partition-contiguous stores back to HBM.
"""

import numpy as np

import concourse.bacc as bacc
import concourse.bass as bass
import concourse.mybir as mybir
import concourse.tile as tile
from concourse.bass_utils import run_bass_kernel_spmd

B, H, W, C = 16, 64, 64, 64
K = C * (C - 1) // 2  # 2016
N_CORES = 8
BP = B // N_CORES  # batch rows per core
P = BP * H         # 128 SBUF partitions
G_TOTAL = W        # position groups per partition
G = 8              # groups processed per iteration
FP = mybir.dt.float32

_nc_cache = None


def build_bass() -> bass.Bass:
    # Bacc (not plain Bass): its compile() pipeline runs
    # generate_event_semaphores, which splits multi-wait instructions to
    # satisfy the TRN2 1-wait-per-instruction codegen limit.
    nc = bacc.Bacc(
        "TRN2",
        target_bir_lowering=False,
        debug=False,
        num_devices=N_CORES,
    )
    x = nc.dram_tensor("x", [P, G_TOTAL, C], FP, kind="ExternalInput")
    y = nc.dram_tensor("y", [P, G_TOTAL, K], FP, kind="ExternalOutput")

    with tile.TileContext(nc) as tc:
        with (
            tc.tile_pool(name="xin", bufs=4) as xpool,
            tc.tile_pool(name="out", bufs=2) as opool,
        ):
            for it in range(G_TOTAL // G):
                xt = xpool.tile([P, G, C], FP)
                # Input loads ride the ACT HWDGE ring so they never queue
                # behind the 8 MB output stores on the SP ring.
                nc.scalar.dma_start(out=xt[:], in_=x[:, it * G : (it + 1) * G, :])

                ot = opool.tile([P, G, K], FP)
                ro = 0
                for i in range(C - 1):
                    w = C - 1 - i
                    a = xt[:, :, i : i + 1].broadcast_to([P, G, w])
                    b = xt[:, :, i + 1 : C]
                    nc.vector.tensor_mul(ot[:, :, ro : ro + w], a, b)
                    ro += w

                nc.sync.dma_start(out=y[:, it * G : (it + 1) * G, :], in_=ot[:])

    nc.finalize()
    return nc


def make_in_maps(x: np.ndarray) -> list[dict[str, np.ndarray]]:
    x = np.ascontiguousarray(x, dtype=np.float32)
    return [
        {"x": x[c * BP : (c + 1) * BP].reshape(P, G_TOTAL, C)} for c in range(N_CORES)
    ]


def kernel(**inputs: np.ndarray) -> np.ndarray:
    global _nc_cache
    if _nc_cache is None:
        _nc_cache = build_bass()
    res = run_bass_kernel_spmd(
        _nc_cache, make_in_maps(inputs["inputs"]), list(range(N_CORES))
    ).results
    return np.concatenate(
        [res[c]["y"].reshape(BP, H, W, K) for c in range(N_CORES)], axis=0
    )
